# revision 57
# baseline (speedup 1.0000x reference)
"""Causal self-attention kernel for 8 Trainium2 NeuronCores.

Problem (hardcoded): x [4, 2048, 1024], torch-style Linear weights
W_q/W_k/W_v/W_o [1024, 1024], b_o [1024]; 16 heads, head_dim 64,
causal softmax attention, out = attn(x) @ W_o.T + b_o.

Sharding: 8 cores = 4 batches x 2 head-groups (8 heads each).
Each core computes a partial output  y_g @ W_o[:, g].T  for its batch;
the host sums the two head-group partials and adds b_o (unshard step).

Per-core pipeline:
  - QKV projections in fp8e4 DoubleRow (0.5 cyc/row) with host-side
    residual compensation: x = x8 + xr8, 64*W = w8 + wr8; three DoubleRow
    terms approximate the fp32 product to ~1e-3. The 64x weight scale is
    divided back out by the exp scale (q/k) and folded into W_o (v).
    q/k land as qT/kT [dq, T] fp16; v as [T, dv] fp16 with an appended
    ones-column (v_aug) for the softmax denominator.
  - scores per head/row-block r: S^T[k, q] for q >= 128r (exact causal),
    exp on ACT into fp16 expP tiles, triangular mask-mul on the diagonal
    block (Pool). Short rounds (r >= 12) pack two PSUM windows per tile.
  - PV flipped: ya[128q, 65] = sum_r expP_r^T @ v_aug (65-cycle blocks
    instead of 128), reciprocal + per-partition tensor_scalar normalize
    on DVE, one PE transpose per q-tile PAIR into yT, out-proj yT @ woT.
  - Software pipelining: a chase schedule places pv/transpose units of
    head h at specific score rounds of head h+1 (matching ACT's exp lag),
    and a deadline-gated background queue spreads the projection work so
    PE keeps running while ACT computes exps.
"""

import collections

import numpy as np

import concourse.bass as bass
import concourse.tile as tile
import concourse.mybir as mybir
from concourse import bacc
from concourse import bass_utils

T = 2048
D = 1024
HPC = 8            # heads per core
DH = 64
DQ = HPC * DH      # 512, per-core projection width
NT = T // 128      # 16 row tiles
NJ = DQ // 128     # 4 dq tiles

F32 = mybir.dt.float32
F16 = mybir.dt.float16
F8 = mybir.dt.float8e4
EXP = mybir.ActivationFunctionType.Exp
MULT = mybir.AluOpType.mult
DIV = mybir.AluOpType.divide
DR = mybir.MatmulPerfMode.DoubleRow

# q/k/v weights are host-scaled by 64 so fp8e4 sees normal-range values;
# scores pick up 64*64 which the exp scale divides back out, and the v-side
# 64 is folded into W_o on the host.
WSCALE = 64.0
EXP_SCALE = 0.125 / (WSCALE * WSCALE)

TRACE = False
LAST = None        # BassKernelResults of the most recent run

TRIMASK = np.triu(np.ones((128, 128), dtype=np.float16))
IDENT = np.eye(128, dtype=np.float16)


def _body(tc):
    nc = tc.nc
    x8_d = nc.dram_tensor("x8", (128, 4, 2, T), F8, kind="ExternalInput").ap()
    xr8_d = nc.dram_tensor("xr8", (128, 4, 2, T), F8, kind="ExternalInput").ap()
    w8_d = {}
    wr8_d = {}
    for wname in ("wq", "wk", "wv"):
        w8_d[wname] = nc.dram_tensor(
            f"{wname}8", (128, 4, 2, DQ), F8, kind="ExternalInput").ap()
        wr8_d[wname] = nc.dram_tensor(
            f"{wname}r8", (128, 4, 2, DQ), F8, kind="ExternalInput").ap()
    wo_d = nc.dram_tensor("wo", (128, NJ, D), F16, kind="ExternalInput").ap()
    tm_d = nc.dram_tensor("trimask", (128, 128), F16, kind="ExternalInput").ap()
    id_d = nc.dram_tensor("ident", (128, 128), F16, kind="ExternalInput").ap()
    out_d = nc.dram_tensor("out", (T, D), F16, kind="ExternalOutput").ap()

    with (
        tc.tile_pool(name="persist", bufs=1) as pp,
        tc.tile_pool(name="psum_ya", bufs=2, space="PSUM") as yap,
        tc.tile_pool(name="psum_g", bufs=2, space="PSUM") as gp,
        tc.tile_pool(name="expp", bufs=2) as epool,
        tc.tile_pool(name="small", bufs=12) as sp,
        tc.tile_pool(name="outsb", bufs=3) as op,
    ):
        x8 = pp.tile([128, 4, 2, T], F8, tag="x8")
        xr8 = pp.tile([128, 4, 2, T], F8, tag="xr8")
        w8 = {n: pp.tile([128, 4, 2, DQ], F8, tag=f"{n}8", name=f"{n}8")
              for n in ("wq", "wk", "wv")}
        wr8 = {n: pp.tile([128, 4, 2, DQ], F8, tag=f"{n}r8", name=f"{n}r8")
               for n in ("wq", "wk", "wv")}
        qT = pp.tile([128, NJ, T], F16, tag="qT")
        kT = pp.tile([128, NJ, T], F16, tag="kT")
        v = pp.tile([128, NT, HPC, DH + 1], F16, tag="v")
        yT = pp.tile([128, NJ, T], F16, tag="yT")
        woT = pp.tile([128, NJ, D], F16, tag="woT")
        trim = pp.tile([128, 128], F16, tag="trim")
        ident = pp.tile([128, 128], F16, tag="ident")
        ones = pp.tile([1, DH], F16, tag="ones")

        nc.gpsimd.memset(ones[:], 1.0)
        nc.gpsimd.memset(v[:, :, :, DH:DH + 1], 1.0)
        # warm the ACT exp table while DMAs run
        warm = pp.tile([1, DH], F16, tag="warm")
        nc.scalar.activation(warm[:], ones[:], EXP, scale=1.0)

        # ---- DMA issue order: earliest-needed first, split across the
        # HWDGE (sync) and SWDGE (gpsimd) queues ----
        def cslice(dst, src, c):
            return dst[:, :, :, 512 * c:512 * (c + 1)], \
                   src[:, :, :, 512 * c:512 * (c + 1)]

        # sync (SP HWDGE, free desc-gen): critical path — qk weights + x8.
        # scalar (ACT HWDGE, idle early): xr8 residuals.
        # vector (DVE HWDGE): v/o weights. gpsimd (SWDGE on Pool): masks.
        nc.sync.dma_start(w8["wq"][:], w8_d["wq"])
        nc.sync.dma_start(*cslice(x8, x8_d, 0))
        nc.scalar.dma_start(*cslice(xr8, xr8_d, 0))
        nc.scalar.dma_start(wr8["wq"][:], wr8_d["wq"])
        nc.sync.dma_start(*cslice(x8, x8_d, 1))
        nc.scalar.dma_start(*cslice(xr8, xr8_d, 1))
        nc.sync.dma_start(*cslice(x8, x8_d, 2))
        nc.scalar.dma_start(*cslice(xr8, xr8_d, 2))
        nc.sync.dma_start(*cslice(x8, x8_d, 3))
        nc.scalar.dma_start(*cslice(xr8, xr8_d, 3))
        nc.sync.dma_start(w8["wk"][:], w8_d["wk"])
        nc.sync.dma_start(wr8["wk"][:], wr8_d["wk"])
        nc.gpsimd.dma_start(trim[:], tm_d)
        nc.gpsimd.dma_start(ident[:], id_d)
        nc.sync.dma_start(w8["wv"][:], w8_d["wv"])
        nc.sync.dma_start(wr8["wv"][:], wr8_d["wv"])
        nc.scalar.dma_start(woT[:], wo_d)

        # ---- two-tier filler queues: (est_cycles, emit_fn, kind) ----
        # urgent: pv/tp/outproj units (tight deadlines: ep-buffer release,
        # tail overlap); background: projection units (loose deadlines).
        urgent = collections.deque()
        bg = collections.deque()
        # earliest head during whose scores a bg kind may be pulled
        # (just-in-time: qk_j completes during sc(2j-1), vp_j during sc(2j))
        allow = {"qk0": 0, "vp0": 0, "qk1": 0, "vp1": 2,
                 "qk2": 2, "vp2": 4, "qk3": 4, "vp3": 6}

        def fill(budget, h=99):
            while budget > 0 and urgent:
                cost, fn, _ = urgent.popleft()
                fn()
                budget -= cost
            while budget > 0 and bg and allow[bg[0][2]] <= h:
                cost, fn, _ = bg.popleft()
                fn()
                budget -= cost

        def drain(kind):
            for q in (urgent, bg):
                while any(u[2] == kind for u in q):
                    _, fn, _ = q.popleft()
                    fn()

        def drain_all():
            for q in (urgent, bg):
                while q:
                    _, fn, _ = q.popleft()
                    fn()

        # QK projection unit: one (weight, j, c) fp8 DoubleRow group.
        # (w8+wr8)(x8+xr8) ~ w8 x8 + w8 xr8 + wr8 x8 (the wr8 xr8 cross
        # term is ~1e-3 relative and dropped).
        def qk_unit(wname, dest, j, c):
            def emit():
                ps = gp.tile([128, 512], F32, tag="g", name=f"qk{j}_{c}")
                terms = [(w8[wname], x8), (wr8[wname], x8), (w8[wname], xr8)]
                for ti, (wt, xs) in enumerate(terms):
                    for kp in range(4):
                        nc.tensor.matmul(
                            ps[:],
                            wt[:, kp, :, 128 * j:128 * (j + 1)],
                            xs[:, kp, :, 512 * c:512 * (c + 1)],
                            start=(ti == 0 and kp == 0),
                            stop=(ti == 2 and kp == 3),
                            perf_mode=DR,
                        )
                nc.vector.tensor_copy(dest[:, j, 512 * c:512 * (c + 1)], ps[:])
            return emit

        # V projection unit: one (t, j) fp8 block -> v[:, t, 2j:2j+2, :64]
        def vp_unit(t, j):
            def emit():
                ps = gp.tile([128, 128], F32, tag="g", name=f"vp{t}_{j}")
                terms = [(x8, w8["wv"]), (x8, wr8["wv"]), (xr8, w8["wv"])]
                for ti, (xs, wt) in enumerate(terms):
                    for kp in range(4):
                        nc.tensor.matmul(
                            ps[:],
                            xs[:, kp, :, 128 * t:128 * (t + 1)],
                            wt[:, kp, :, 128 * j:128 * (j + 1)],
                            start=(ti == 0 and kp == 0),
                            stop=(ti == 2 and kp == 3),
                            perf_mode=DR,
                        )
                nc.vector.tensor_copy(
                    v[:, t, 2 * j:2 * j + 2, 0:DH],
                    ps[:].rearrange("p (h d) -> p h d", h=2),
                )
            return emit

        # deadline order: qk0 < vp0 (pv0 @ sc1) < qk1 (sc2) < vp1 (pv2 @ sc3)
        # < qk2 (sc4) < vp2 < qk3 (sc6) < vp3
        # qk0 is emitted directly at kernel start (k-units chased into
        # sc(0) rounds just before each needs them)
        for t in range(NT):
            bg.append((768, vp_unit(t, 0), "vp0"))
        for j in range(1, NJ):
            for c in range(4):
                bg.append((3072, qk_unit("wq", qT, j, c), f"qk{j}"))
                bg.append((3072, qk_unit("wk", kT, j, c), f"qk{j}"))
            for t in range(NT):
                bg.append((768, vp_unit(t, j), f"vp{j}"))

        # ---- scores + exp for one head ----
        # chase: {round r: [unit, ...]} — units placed after round r's
        # windows (deterministic pacing against ACT's exp pipeline).
        expps = {}

        def scores_head(h, chase=None):
            hp = (h % 2) * DH
            hj = h // 2
            eps = [None] * NT
            share = {}
            for r in range(NT):
                lo = 128 * r
                ep = epool.tile([128, T - lo], F16, tag=f"e{r}",
                                name=f"e{r}_h{h}")
                eps[r] = ep
                bounds = [(lo, min(lo + 1024, T))]
                if lo + 1024 < T:
                    bounds.append((lo + 1024, T))
                first = True
                for (wlo, whi) in bounds:
                    # short rounds (r>=12) pack two windows per stp tile
                    # (subtile ranges) to double the pipeline depth there
                    if r >= 12:
                        if r % 2 == 0:
                            share["tile"] = stp.tile(
                                [128, 1024], F32, tag="st",
                                name=f"stsh{r}_h{h}")
                            share["off"] = 0
                        st = share["tile"][:, share["off"]:
                                           share["off"] + (whi - wlo)]
                        share["off"] += whi - wlo
                    else:
                        st = stp.tile([128, whi - wlo], F32, tag="st",
                                      name=f"st{r}_h{h}")
                    for n0 in range(wlo, whi, 512):
                        n1 = min(n0 + 512, whi)
                        nc.tensor.matmul(
                            st[:, n0 - wlo:n1 - wlo],
                            kT[hp:hp + DH, hj, lo:lo + 128],
                            qT[hp:hp + DH, hj, n0:n1],
                            start=True, stop=True,
                        )
                    nc.scalar.activation(
                        ep[:, wlo - lo:whi - lo], st[:], EXP, scale=EXP_SCALE,
                    )
                    if first:
                        nc.gpsimd.tensor_mul(ep[:, 0:128], ep[:, 0:128],
                                             trim[:])
                        first = False
                if chase:
                    for _, fn, _ in chase.get(r, ()):
                        fn()
                fill(T - lo + 200, h)
            expps[h] = eps

        # ---- flipped PV + normalize for one q-tile ----
        ya_cur = {}
        yn_pend = {}

        def pv_unit(h, i):
            def emit():
                if i % 7 == 0:
                    ya_cur[h] = yap.tile([128, 7, DH + 1], F32, tag="ya",
                                         name=f"ya{h}_{i // 7}")
                sub = ya_cur[h][:, i % 7, :]
                eps = expps[h]
                for r in range(i + 1):
                    nc.tensor.matmul(
                        sub,
                        eps[r][:, 128 * (i - r):128 * (i - r) + 128],
                        v[:, r, h, :],
                        start=(r == 0), stop=(r == i),
                    )
                rec = sp.tile([128, 1], F32, tag="rec", name=f"rec{h}_{i}")
                nc.vector.reciprocal(rec[:], sub[:, DH:DH + 1])
                if i % 2 == 0:
                    yn_pend[(h, i // 2)] = sp.tile(
                        [128, 2, DH], F16, tag="yn", name=f"yn{h}_{i // 2}")
                nc.vector.tensor_scalar(yn_pend[(h, i // 2)][:, i % 2, :],
                                        sub[:, 0:DH], rec[:], None, op0=MULT)
            return (65 * (i + 1) + 100, emit, f"pv{h}")

        # one PE transpose per PAIR of q-tiles: yn [128, 2*64] -> [128, 128]
        def tp_unit(h, m):
            hp = (h % 2) * DH
            hj = h // 2

            def emit():
                yn = yn_pend.pop((h, m))
                tb = gp.tile([128, 128], F16, tag="g", name=f"tb{h}_{m}")
                nc.tensor.transpose(tb[:], yn[:], ident[:])
                for s in range(2):
                    i = 2 * m + s
                    nc.vector.tensor_copy(
                        yT[hp:hp + DH, hj, 128 * i:128 * (i + 1)],
                        tb[64 * s:64 * (s + 1), :])
            return (500, emit, f"pv{h}")

        # ---- out projection for one row tile ----
        def op_unit(i, pool):
            def emit():
                ob = op.tile([128, D], F16, tag="ob", name=f"ob{i}")
                for d in range(2):
                    ps = pool.tile([128, 512], F32, tag="opg", name=f"op{i}_{d}")
                    for j in range(NJ):
                        nc.tensor.matmul(
                            ps[:],
                            yT[:, j, 128 * i:128 * (i + 1)],
                            woT[:, j, 512 * d:512 * (d + 1)],
                            start=(j == 0), stop=(j == 3),
                        )
                    if d == 0:
                        nc.scalar.copy(ob[:, 0:512], ps[:])
                    else:
                        nc.vector.tensor_copy(ob[:, 512:D], ps[:])
                    q = nc.sync if (2 * i + d) % 2 == 0 else nc.scalar
                    q.dma_start(out_d[128 * i:128 * (i + 1),
                                      512 * d:512 * (d + 1)],
                                ob[:, 512 * d:512 * (d + 1)])
            return (4096, emit, "op")

        # ---- main schedule ----
        # pv(h) units run during sc(h+1), placed at round ~i+2 (exp(h, i)
        # completes while sc(h+1) streams; ACT lags sc-mm by ~1 window).
        stp_ctx = tc.tile_pool(name="psum_st", bufs=2, space="PSUM")
        stp = stp_ctx.__enter__()

        # head 0 lead-in: all wq units, then k-c0; later k-chunks chased
        # into sc(0) right before the score rows that read them.
        for c in range(4):
            qk_unit("wq", qT, 0, c)()
        qk_unit("wk", kT, 0, 0)()
        chase = {3: [(3072, qk_unit("wk", kT, 0, 1), "qk0")],
                 7: [(3072, qk_unit("wk", kT, 0, 2), "qk0")],
                 11: [(3072, qk_unit("wk", kT, 0, 3), "qk0")]}
        spill = {0: [], 1: []}
        for h in range(HPC):
            drain(f"qk{h // 2}")
            scores_head(h, chase)
            drain(f"vp{h // 2}")
            # Build next head's chase. The last pv/tp units of head h are
            # deferred past the NEXT head boundary (spill) so the boundary
            # gap goes to scores-mm (feeding ACT) instead of pv work.
            nxt = collections.defaultdict(list)
            if h < HPC - 1:
                nxt[0].extend(spill[0])
                nxt[1].extend(spill[1])
                spill = {0: [], 1: []}
                for i in range(NT - 2):
                    nxt[min(i + 2, NT - 3)].append(pv_unit(h, i))
                for m in range(NT // 2 - 1):
                    nxt[min(2 * m + 5, NT - 3)].append(tp_unit(h, m))
                if h < HPC - 2:
                    spill[0].extend([pv_unit(h, NT - 2), pv_unit(h, NT - 1)])
                    spill[1].append(tp_unit(h, NT // 2 - 1))
                else:
                    for i in (NT - 2, NT - 1):
                        nxt[NT - 1].append(pv_unit(h, i))
                    nxt[NT - 1].append(tp_unit(h, NT // 2 - 1))
            chase = nxt
        drain_all()
        stp_ctx.__exit__(None, None, None)

        # tail: pv(7)/tp(7) paced against exp(7); outproj (own psum pool,
        # using the banks freed by stp) fills the stalls.
        with tc.tile_pool(name="psum_op", bufs=4, space="PSUM") as opp:
            h7 = HPC - 1
            for m in range(NT // 2):
                pv_unit(h7, 2 * m)[1]()
                pv_unit(h7, 2 * m + 1)[1]()
                if m >= 1:
                    tp_unit(h7, m - 1)[1]()
                if m >= 2:
                    op_unit(2 * (m - 2), opp)[1]()
                    op_unit(2 * (m - 2) + 1, opp)[1]()
            tp_unit(h7, NT // 2 - 1)[1]()
            for i in range(NT - 4, NT):
                op_unit(i, opp)[1]()


def build_nc():
    nc = bacc.Bacc("TRN2", target_bir_lowering=False, debug=False)
    with tile.TileContext(nc) as tc:
        _body(tc)
    nc.compile()
    return nc


_nc_cache = None


def _get_nc():
    global _nc_cache
    if _nc_cache is None:
        _nc_cache = build_nc()
    return _nc_cache


def make_in_maps(x, W_q, W_k, W_v, W_o):
    x = np.asarray(x, dtype=np.float32)
    W_q = np.asarray(W_q, dtype=np.float32)
    W_k = np.asarray(W_k, dtype=np.float32)
    W_v = np.asarray(W_v, dtype=np.float32)
    W_o = np.asarray(W_o, dtype=np.float32)

    F8NP = np.dtype(mybir.dt.np(F8))

    def kblocks(m, nb):
        # [D, N] -> [128, nb, N] with row index d = c*128 + p -> [p, c, n]
        return np.ascontiguousarray(
            m.reshape(nb, 128, m.shape[1]).transpose(1, 0, 2))

    def split8(m):
        # [128, 8, N] f32 -> fp8 value + fp8 residual, pair-packed
        # [128, 4, 2, N]
        m = m.reshape(128, 4, 2, -1)
        m8 = m.astype(F8NP)
        r8 = (m - m8.astype(np.float32)).astype(F8NP)
        return np.ascontiguousarray(m8), np.ascontiguousarray(r8)

    in_maps = []
    for core in range(8):
        b, g = divmod(core, 2)
        sl = slice(DQ * g, DQ * (g + 1))
        xT = np.ascontiguousarray(x[b].T)          # [D, T]
        x8, xr8 = split8(kblocks(xT, 8))
        im = {"x8": x8, "xr8": xr8, "trimask": TRIMASK, "ident": IDENT,
              "wo": kblocks(np.ascontiguousarray(W_o[:, sl].T) / WSCALE, 4)
              .astype(np.float16)}
        for wname, W in (("wq", W_q), ("wk", W_k), ("wv", W_v)):
            w8, wr8 = split8(kblocks(
                np.ascontiguousarray(W[sl].T) * WSCALE, 8))
            im[f"{wname}8"] = w8
            im[f"{wname}r8"] = wr8
        in_maps.append(im)
    return in_maps


def kernel(x, W_q, W_k, W_v, W_o, b_o):
    global LAST
    nc = _get_nc()
    in_maps = make_in_maps(x, W_q, W_k, W_v, W_o)
    res = bass_utils.run_bass_kernel_spmd(
        nc, in_maps, core_ids=list(range(8)), trace=TRACE
    )
    LAST = res
    parts = [np.asarray(res.results[c]["out"], dtype=np.float32)
             for c in range(8)]
    b_o = np.asarray(b_o, dtype=np.float32)
    out = np.stack([parts[2 * b] + parts[2 * b + 1] for b in range(4)])
    out += b_o[None, None, :]
    return out.astype(np.float32)


# revision 58
# speedup vs baseline: 1.0005x; 1.0005x over previous
"""Causal self-attention kernel for 8 Trainium2 NeuronCores.

Problem (hardcoded): x [4, 2048, 1024], torch-style Linear weights
W_q/W_k/W_v/W_o [1024, 1024], b_o [1024]; 16 heads, head_dim 64,
causal softmax attention, out = attn(x) @ W_o.T + b_o.

Sharding: 8 cores = 4 batches x 2 head-groups (8 heads each).
Each core computes a partial output  y_g @ W_o[:, g].T  for its batch;
the host sums the two head-group partials and adds b_o (unshard step).

Per-core pipeline:
  - QKV projections in fp8e4 DoubleRow (0.5 cyc/row) with host-side
    residual compensation: x = x8 + xr8, 64*W = w8 + wr8; three DoubleRow
    terms approximate the fp32 product to ~1e-3. The 64x weight scale is
    divided back out by the exp scale (q/k) and folded into W_o (v).
    q/k land as qT/kT [dq, T] fp16; v as [T, dv] fp16 with an appended
    ones-column (v_aug) for the softmax denominator.
  - scores per head/row-block r: S^T[k, q] for q >= 128r (exact causal),
    exp on ACT into fp16 expP tiles, triangular mask-mul on the diagonal
    block (Pool). Short rounds (r >= 12) pack two PSUM windows per tile.
  - PV flipped: ya[128q, 65] = sum_r expP_r^T @ v_aug (65-cycle blocks
    instead of 128), reciprocal + per-partition tensor_scalar normalize
    on DVE, one PE transpose per q-tile PAIR into yT, out-proj yT @ woT.
  - Software pipelining: a chase schedule places pv/transpose units of
    head h at specific score rounds of head h+1 (matching ACT's exp lag),
    and a deadline-gated background queue spreads the projection work so
    PE keeps running while ACT computes exps.
"""

import collections

import numpy as np

import concourse.bass as bass
import concourse.tile as tile
import concourse.mybir as mybir
from concourse import bacc
from concourse import bass_utils

T = 2048
D = 1024
HPC = 8            # heads per core
DH = 64
DQ = HPC * DH      # 512, per-core projection width
NT = T // 128      # 16 row tiles
NJ = DQ // 128     # 4 dq tiles

F32 = mybir.dt.float32
F16 = mybir.dt.float16
F8 = mybir.dt.float8e4
EXP = mybir.ActivationFunctionType.Exp
MULT = mybir.AluOpType.mult
DIV = mybir.AluOpType.divide
DR = mybir.MatmulPerfMode.DoubleRow

# q/k/v weights are host-scaled by 64 so fp8e4 sees normal-range values;
# scores pick up 64*64 which the exp scale divides back out, and the v-side
# 64 is folded into W_o on the host.
WSCALE = 64.0
EXP_SCALE = 0.125 / (WSCALE * WSCALE)

TRACE = False
LAST = None        # BassKernelResults of the most recent run

TRIMASK = np.triu(np.ones((128, 128), dtype=np.float16))
IDENT = np.eye(128, dtype=np.float16)


def _body(tc):
    nc = tc.nc
    x8_d = nc.dram_tensor("x8", (128, 4, 2, T), F8, kind="ExternalInput").ap()
    xr8_d = nc.dram_tensor("xr8", (128, 4, 2, T), F8, kind="ExternalInput").ap()
    w8_d = {}
    wr8_d = {}
    for wname in ("wq", "wk", "wv"):
        w8_d[wname] = nc.dram_tensor(
            f"{wname}8", (128, 4, 2, DQ), F8, kind="ExternalInput").ap()
        wr8_d[wname] = nc.dram_tensor(
            f"{wname}r8", (128, 4, 2, DQ), F8, kind="ExternalInput").ap()
    wo_d = nc.dram_tensor("wo", (128, NJ, D), F16, kind="ExternalInput").ap()
    tm_d = nc.dram_tensor("trimask", (128, 128), F16, kind="ExternalInput").ap()
    id_d = nc.dram_tensor("ident", (128, 128), F16, kind="ExternalInput").ap()
    out_d = nc.dram_tensor("out", (T, D), F16, kind="ExternalOutput").ap()

    with (
        tc.tile_pool(name="persist", bufs=1) as pp,
        tc.tile_pool(name="psum_ya", bufs=2, space="PSUM") as yap,
        tc.tile_pool(name="psum_g", bufs=2, space="PSUM") as gp,
        tc.tile_pool(name="expp", bufs=2) as epool,
        tc.tile_pool(name="small", bufs=12) as sp,
        tc.tile_pool(name="outsb", bufs=3) as op,
    ):
        x8 = pp.tile([128, 4, 2, T], F8, tag="x8")
        xr8 = pp.tile([128, 4, 2, T], F8, tag="xr8")
        w8 = {n: pp.tile([128, 4, 2, DQ], F8, tag=f"{n}8", name=f"{n}8")
              for n in ("wq", "wk", "wv")}
        wr8 = {n: pp.tile([128, 4, 2, DQ], F8, tag=f"{n}r8", name=f"{n}r8")
               for n in ("wq", "wk", "wv")}
        qT = pp.tile([128, NJ, T], F16, tag="qT")
        kT = pp.tile([128, NJ, T], F16, tag="kT")
        v = pp.tile([128, NT, HPC, DH + 1], F16, tag="v")
        yT = pp.tile([128, NJ, T], F16, tag="yT")
        woT = pp.tile([128, NJ, D], F16, tag="woT")
        trim = pp.tile([128, 128], F16, tag="trim")
        ident = pp.tile([128, 128], F16, tag="ident")
        ones = pp.tile([1, DH], F16, tag="ones")

        nc.gpsimd.memset(ones[:], 1.0)
        nc.gpsimd.memset(v[:, :, :, DH:DH + 1], 1.0)
        # warm the ACT exp table while DMAs run
        warm = pp.tile([1, DH], F16, tag="warm")
        nc.scalar.activation(warm[:], ones[:], EXP, scale=1.0)

        # ---- DMA issue order: earliest-needed first, split across the
        # HWDGE (sync) and SWDGE (gpsimd) queues ----
        def cslice(dst, src, c):
            return dst[:, :, :, 512 * c:512 * (c + 1)], \
                   src[:, :, :, 512 * c:512 * (c + 1)]

        # sync (SP HWDGE, free desc-gen): critical path — qk weights + x8.
        # scalar (ACT HWDGE, idle early): xr8 residuals.
        # vector (DVE HWDGE): v/o weights. gpsimd (SWDGE on Pool): masks.
        nc.sync.dma_start(w8["wq"][:], w8_d["wq"])
        nc.sync.dma_start(*cslice(x8, x8_d, 0))
        nc.scalar.dma_start(*cslice(xr8, xr8_d, 0))
        nc.scalar.dma_start(wr8["wq"][:], wr8_d["wq"])
        nc.sync.dma_start(*cslice(x8, x8_d, 1))
        nc.scalar.dma_start(*cslice(xr8, xr8_d, 1))
        nc.sync.dma_start(*cslice(x8, x8_d, 2))
        nc.scalar.dma_start(*cslice(xr8, xr8_d, 2))
        nc.sync.dma_start(*cslice(x8, x8_d, 3))
        nc.scalar.dma_start(*cslice(xr8, xr8_d, 3))
        nc.sync.dma_start(w8["wk"][:], w8_d["wk"])
        nc.sync.dma_start(wr8["wk"][:], wr8_d["wk"])
        nc.gpsimd.dma_start(trim[:], tm_d)
        nc.gpsimd.dma_start(ident[:], id_d)
        nc.sync.dma_start(w8["wv"][:], w8_d["wv"])
        nc.sync.dma_start(wr8["wv"][:], wr8_d["wv"])
        nc.scalar.dma_start(woT[:], wo_d)

        # ---- two-tier filler queues: (est_cycles, emit_fn, kind) ----
        # urgent: pv/tp/outproj units (tight deadlines: ep-buffer release,
        # tail overlap); background: projection units (loose deadlines).
        urgent = collections.deque()
        bg = collections.deque()
        # earliest head during whose scores a bg kind may be pulled
        # (just-in-time: qk_j completes during sc(2j-1), vp_j during sc(2j))
        allow = {"qk0": 0, "vp0": 0, "qk1": 0, "vp1": 2,
                 "qk2": 2, "vp2": 4, "qk3": 4, "vp3": 6}

        def fill(budget, h=99):
            while budget > 0 and urgent:
                cost, fn, _ = urgent.popleft()
                fn()
                budget -= cost
            while budget > 0 and bg and allow[bg[0][2]] <= h:
                cost, fn, _ = bg.popleft()
                fn()
                budget -= cost

        def drain(kind):
            for q in (urgent, bg):
                while any(u[2] == kind for u in q):
                    _, fn, _ = q.popleft()
                    fn()

        def drain_all():
            for q in (urgent, bg):
                while q:
                    _, fn, _ = q.popleft()
                    fn()

        # QK projection unit: one (weight, j, c) fp8 DoubleRow group.
        # (w8+wr8)(x8+xr8) ~ w8 x8 + w8 xr8 + wr8 x8 (the wr8 xr8 cross
        # term is ~1e-3 relative and dropped).
        def qk_unit(wname, dest, j, c):
            def emit():
                ps = gp.tile([128, 512], F32, tag="g", name=f"qk{j}_{c}")
                terms = [(w8[wname], x8), (wr8[wname], x8), (w8[wname], xr8)]
                for ti, (wt, xs) in enumerate(terms):
                    for kp in range(4):
                        nc.tensor.matmul(
                            ps[:],
                            wt[:, kp, :, 128 * j:128 * (j + 1)],
                            xs[:, kp, :, 512 * c:512 * (c + 1)],
                            start=(ti == 0 and kp == 0),
                            stop=(ti == 2 and kp == 3),
                            perf_mode=DR,
                        )
                nc.vector.tensor_copy(dest[:, j, 512 * c:512 * (c + 1)], ps[:])
            return emit

        # V projection unit: one (t, j) fp8 block -> v[:, t, 2j:2j+2, :64]
        def vp_unit(t, j):
            def emit():
                ps = gp.tile([128, 128], F32, tag="g", name=f"vp{t}_{j}")
                terms = [(x8, w8["wv"]), (x8, wr8["wv"]), (xr8, w8["wv"])]
                for ti, (xs, wt) in enumerate(terms):
                    for kp in range(4):
                        nc.tensor.matmul(
                            ps[:],
                            xs[:, kp, :, 128 * t:128 * (t + 1)],
                            wt[:, kp, :, 128 * j:128 * (j + 1)],
                            start=(ti == 0 and kp == 0),
                            stop=(ti == 2 and kp == 3),
                            perf_mode=DR,
                        )
                nc.vector.tensor_copy(
                    v[:, t, 2 * j:2 * j + 2, 0:DH],
                    ps[:].rearrange("p (h d) -> p h d", h=2),
                )
            return emit

        # deadline order: qk0 < vp0 (pv0 @ sc1) < qk1 (sc2) < vp1 (pv2 @ sc3)
        # < qk2 (sc4) < vp2 < qk3 (sc6) < vp3
        # qk0 is emitted directly at kernel start (k-units chased into
        # sc(0) rounds just before each needs them)
        for t in range(NT):
            bg.append((768, vp_unit(t, 0), "vp0"))
        for j in range(1, NJ):
            for c in range(4):
                bg.append((3072, qk_unit("wq", qT, j, c), f"qk{j}"))
                bg.append((3072, qk_unit("wk", kT, j, c), f"qk{j}"))
            for t in range(NT):
                bg.append((768, vp_unit(t, j), f"vp{j}"))

        # ---- scores + exp for one head ----
        # chase: {round r: [unit, ...]} — units placed after round r's
        # windows (deterministic pacing against ACT's exp pipeline).
        expps = {}

        def scores_head(h, chase=None):
            hp = (h % 2) * DH
            hj = h // 2
            eps = [None] * NT
            share = {}
            for r in range(NT):
                lo = 128 * r
                ep = epool.tile([128, T - lo], F16, tag=f"e{r}",
                                name=f"e{r}_h{h}")
                eps[r] = ep
                bounds = [(lo, min(lo + 1024, T))]
                if lo + 1024 < T:
                    bounds.append((lo + 1024, T))
                first = True
                for (wlo, whi) in bounds:
                    # short rounds (r>=12) pack two windows per stp tile
                    # (subtile ranges) to double the pipeline depth there
                    if r >= 12:
                        if r % 2 == 0:
                            share["tile"] = stp.tile(
                                [128, 1024], F32, tag="st",
                                name=f"stsh{r}_h{h}")
                            share["off"] = 0
                        st = share["tile"][:, share["off"]:
                                           share["off"] + (whi - wlo)]
                        share["off"] += whi - wlo
                    else:
                        st = stp.tile([128, whi - wlo], F32, tag="st",
                                      name=f"st{r}_h{h}")
                    for n0 in range(wlo, whi, 512):
                        n1 = min(n0 + 512, whi)
                        nc.tensor.matmul(
                            st[:, n0 - wlo:n1 - wlo],
                            kT[hp:hp + DH, hj, lo:lo + 128],
                            qT[hp:hp + DH, hj, n0:n1],
                            start=True, stop=True,
                        )
                    nc.scalar.activation(
                        ep[:, wlo - lo:whi - lo], st[:], EXP, scale=EXP_SCALE,
                    )
                    if first:
                        nc.gpsimd.tensor_mul(ep[:, 0:128], ep[:, 0:128],
                                             trim[:])
                        first = False
                if chase:
                    for _, fn, _ in chase.get(r, ()):
                        fn()
                fill(T - lo + 200, h)
            expps[h] = eps

        # ---- flipped PV + normalize for one q-tile ----
        ya_cur = {}
        yn_pend = {}

        def pv_unit(h, i):
            def emit():
                if i % 4 == 0:
                    ya_cur[h] = yap.tile([128, 4, DH + 1], F32, tag="ya",
                                         name=f"ya{h}_{i // 4}")
                sub = ya_cur[h][:, i % 4, :]
                eps = expps[h]
                for r in range(i + 1):
                    nc.tensor.matmul(
                        sub,
                        eps[r][:, 128 * (i - r):128 * (i - r) + 128],
                        v[:, r, h, :],
                        start=(r == 0), stop=(r == i),
                    )
                rec = sp.tile([128, 1], F32, tag="rec", name=f"rec{h}_{i}")
                nc.vector.reciprocal(rec[:], sub[:, DH:DH + 1])
                if i % 2 == 0:
                    yn_pend[(h, i // 2)] = sp.tile(
                        [128, 2, DH], F16, tag="yn", name=f"yn{h}_{i // 2}")
                nc.vector.tensor_scalar(yn_pend[(h, i // 2)][:, i % 2, :],
                                        sub[:, 0:DH], rec[:], None, op0=MULT)
            return (65 * (i + 1) + 100, emit, f"pv{h}")

        # one PE transpose per PAIR of q-tiles: yn [128, 2*64] -> [128, 128]
        def tp_unit(h, m):
            hp = (h % 2) * DH
            hj = h // 2

            def emit():
                yn = yn_pend.pop((h, m))
                tb = gp.tile([128, 128], F16, tag="g", name=f"tb{h}_{m}")
                nc.tensor.transpose(tb[:], yn[:], ident[:])
                for s in range(2):
                    i = 2 * m + s
                    nc.vector.tensor_copy(
                        yT[hp:hp + DH, hj, 128 * i:128 * (i + 1)],
                        tb[64 * s:64 * (s + 1), :])
            return (500, emit, f"pv{h}")

        # ---- out projection for one row tile ----
        def op_unit(i, pool):
            def emit():
                ob = op.tile([128, D], F16, tag="ob", name=f"ob{i}")
                for d in range(2):
                    ps = pool.tile([128, 512], F32, tag="opg", name=f"op{i}_{d}")
                    for j in range(NJ):
                        nc.tensor.matmul(
                            ps[:],
                            yT[:, j, 128 * i:128 * (i + 1)],
                            woT[:, j, 512 * d:512 * (d + 1)],
                            start=(j == 0), stop=(j == 3),
                        )
                    if d == 0:
                        nc.scalar.copy(ob[:, 0:512], ps[:])
                    else:
                        nc.vector.tensor_copy(ob[:, 512:D], ps[:])
                    q = nc.sync if (2 * i + d) % 2 == 0 else nc.scalar
                    q.dma_start(out_d[128 * i:128 * (i + 1),
                                      512 * d:512 * (d + 1)],
                                ob[:, 512 * d:512 * (d + 1)])
            return (4096, emit, "op")

        # ---- main schedule ----
        # pv(h) units run during sc(h+1), placed at round ~i+2 (exp(h, i)
        # completes while sc(h+1) streams; ACT lags sc-mm by ~1 window).
        stp_ctx = tc.tile_pool(name="psum_st", bufs=2, space="PSUM")
        stp = stp_ctx.__enter__()

        # head 0 lead-in: all wq units, then k-c0; later k-chunks chased
        # into sc(0) right before the score rows that read them.
        for c in range(4):
            qk_unit("wq", qT, 0, c)()
        qk_unit("wk", kT, 0, 0)()
        chase = {3: [(3072, qk_unit("wk", kT, 0, 1), "qk0")],
                 7: [(3072, qk_unit("wk", kT, 0, 2), "qk0")],
                 11: [(3072, qk_unit("wk", kT, 0, 3), "qk0")]}
        spill = {0: [], 1: []}
        for h in range(HPC):
            drain(f"qk{h // 2}")
            scores_head(h, chase)
            drain(f"vp{h // 2}")
            # Build next head's chase. The last pv/tp units of head h are
            # deferred past the NEXT head boundary (spill) so the boundary
            # gap goes to scores-mm (feeding ACT) instead of pv work.
            nxt = collections.defaultdict(list)
            if h < HPC - 1:
                nxt[0].extend(spill[0])
                nxt[1].extend(spill[1])
                spill = {0: [], 1: []}
                for i in range(NT - 2):
                    nxt[min(i + 2, NT - 3)].append(pv_unit(h, i))
                for m in range(NT // 2 - 1):
                    nxt[min(2 * m + 5, NT - 3)].append(tp_unit(h, m))
                if h < HPC - 2:
                    spill[0].extend([pv_unit(h, NT - 2), pv_unit(h, NT - 1)])
                    spill[1].append(tp_unit(h, NT // 2 - 1))
                else:
                    for i in (NT - 2, NT - 1):
                        nxt[NT - 1].append(pv_unit(h, i))
                    nxt[NT - 1].append(tp_unit(h, NT // 2 - 1))
            chase = nxt
        drain_all()
        stp_ctx.__exit__(None, None, None)

        # tail: pv(7)/tp(7) paced against exp(7); outproj (own psum pool,
        # using the banks freed by stp) fills the stalls.
        with tc.tile_pool(name="psum_op", bufs=4, space="PSUM") as opp:
            h7 = HPC - 1
            for m in range(NT // 2):
                pv_unit(h7, 2 * m)[1]()
                pv_unit(h7, 2 * m + 1)[1]()
                if m >= 1:
                    tp_unit(h7, m - 1)[1]()
                if m >= 2:
                    op_unit(2 * (m - 2), opp)[1]()
                    op_unit(2 * (m - 2) + 1, opp)[1]()
            tp_unit(h7, NT // 2 - 1)[1]()
            for i in range(NT - 4, NT):
                op_unit(i, opp)[1]()


def build_nc():
    nc = bacc.Bacc("TRN2", target_bir_lowering=False, debug=False)
    with tile.TileContext(nc) as tc:
        _body(tc)
    nc.compile()
    return nc


_nc_cache = None


def _get_nc():
    global _nc_cache
    if _nc_cache is None:
        _nc_cache = build_nc()
    return _nc_cache


def make_in_maps(x, W_q, W_k, W_v, W_o):
    x = np.asarray(x, dtype=np.float32)
    W_q = np.asarray(W_q, dtype=np.float32)
    W_k = np.asarray(W_k, dtype=np.float32)
    W_v = np.asarray(W_v, dtype=np.float32)
    W_o = np.asarray(W_o, dtype=np.float32)

    F8NP = np.dtype(mybir.dt.np(F8))

    def kblocks(m, nb):
        # [D, N] -> [128, nb, N] with row index d = c*128 + p -> [p, c, n]
        return np.ascontiguousarray(
            m.reshape(nb, 128, m.shape[1]).transpose(1, 0, 2))

    def split8(m):
        # [128, 8, N] f32 -> fp8 value + fp8 residual, pair-packed
        # [128, 4, 2, N]
        m = m.reshape(128, 4, 2, -1)
        m8 = m.astype(F8NP)
        r8 = (m - m8.astype(np.float32)).astype(F8NP)
        return np.ascontiguousarray(m8), np.ascontiguousarray(r8)

    in_maps = []
    for core in range(8):
        b, g = divmod(core, 2)
        sl = slice(DQ * g, DQ * (g + 1))
        xT = np.ascontiguousarray(x[b].T)          # [D, T]
        x8, xr8 = split8(kblocks(xT, 8))
        im = {"x8": x8, "xr8": xr8, "trimask": TRIMASK, "ident": IDENT,
              "wo": kblocks(np.ascontiguousarray(W_o[:, sl].T) / WSCALE, 4)
              .astype(np.float16)}
        for wname, W in (("wq", W_q), ("wk", W_k), ("wv", W_v)):
            w8, wr8 = split8(kblocks(
                np.ascontiguousarray(W[sl].T) * WSCALE, 8))
            im[f"{wname}8"] = w8
            im[f"{wname}r8"] = wr8
        in_maps.append(im)
    return in_maps


def kernel(x, W_q, W_k, W_v, W_o, b_o):
    global LAST
    nc = _get_nc()
    in_maps = make_in_maps(x, W_q, W_k, W_v, W_o)
    res = bass_utils.run_bass_kernel_spmd(
        nc, in_maps, core_ids=list(range(8)), trace=TRACE
    )
    LAST = res
    parts = [np.asarray(res.results[c]["out"], dtype=np.float32)
             for c in range(8)]
    b_o = np.asarray(b_o, dtype=np.float32)
    out = np.stack([parts[2 * b] + parts[2 * b + 1] for b in range(4)])
    out += b_o[None, None, :]
    return out.astype(np.float32)
